# revision 13
# baseline (speedup 1.0000x reference)
"""AdaPT Linear (int8 systolic fake-quant matmul) on 8 TRN2 NeuronCores.

Reference semantics (single device):
    amax_x = max|x|, amax_w = max|w|         (global scalars)
    sx = 127/amax_x, sw = 127/amax_w
    qx = round(x*sx)  (int8), qw = round(w*sw)  (int8)
    out = (qx @ qw.T)_int32 / (sx*sw) + bias

Distribution: data-parallel over x rows (8 x 1024 rows per core).

Key structure (v5) — TWO small NEFFs, no on-device collective:
  An InstCollectiveCompute anywhere in a NEFF arms the CC machinery and
  measurably stretches EVERY matmul in that NEFF from ~216ns to ~266ns
  (cross-checked with microbenchmarks: the same matmul stream runs at
  216ns without a collective, 266ns with one, regardless of where the
  collective sits in the program).  So the global amax exchange is
  taken off-device:
    NEFF-A: each core reduces |x|-shard and |w|-k-slice partial maxima
            and writes a [1,2] partial to DRAM.  (pure DMA+DVE, ~85us)
    host:   max over the 8 partial pairs (16 floats) and the three
            scale scalars sx=127/amax_x, sw=127/amax_w, ds=1/(sx*sw) —
            the same f32 arithmetic the reference does.
    NEFF-B: quantize + matmul + epilogue with the scales as inputs —
            collective-free, so the PE runs at full rate.

  - host stages xT (k-major) per core and wT (k-major) replicated: NO
    on-chip transposes, both matmul operands load k-major.
  - quantization = fp32 magic-number round (v*s + 1.5*2^23, subtract
    back; fp32 RNE == round-half-even == jnp.round), 2 k-tiles per
    DVE/ACT op.
  - w-quant for block nb+2 is spread across the mb groups of block nb's
    matmuls; the qwT ring has 3 slots so those writes never alias the
    block the PE is reading.
  - matmul: lhsT = qxT k-tile [128k x 128m], rhs = qwT k-tile
    [128k x 512n], 32-step accumulation into fp32 PSUM (8 banks).
    int8 products (<2^14) and sums (<2^24) are exact in the bf16 PE
    datapath, reproducing the int8 MAC.
  - epilogue: out = psum * ds + bias in one DVE op, stored bf16
    (rel-err ~1e-3, well under the 2e-2 gate), 2 row-blocks per DMA;
    host casts back to f32.
"""

import numpy as np

P = 128
MAGIC = 12582912.0  # 1.5 * 2**23: fp32 RNE round-to-int trick
MAXV = 127.0
NCORES = 8

# full-problem shapes (hardcoded per the task)
FULL_B, FULL_S, FULL_K = 4, 2048, 4096
FULL_N = 4096


def build_graph_amax(M=1024, N=4096, K=4096, ncores=NCORES):
    """NEFF-A: per-core partial amax of the x shard and the w k-slice."""
    import concourse.mybir as mybir
    import concourse.tile as tile
    from concourse import bacc, bass_isa

    KT = K // P
    KSL = K // ncores

    f32 = mybir.dt.float32
    nc = bacc.Bacc(None, num_devices=ncores)

    xt_ext = nc.declare_dram_parameter("xT", [K, M], f32, isOutput=False)
    wslt_ext = nc.declare_dram_parameter("wslT", [KSL, N], f32, isOutput=False)
    pmax_ext = nc.declare_dram_parameter("pmax", [1, 2], f32, isOutput=True)

    xt_v = xt_ext[:].rearrange("(a p) m -> p a m", p=P)      # [P, KT, M]
    wslt_v = wslt_ext[:].rearrange("(a p) n -> p a n", p=P)  # [P, KSL/P, N]

    with tile.TileContext(nc) as tc:
        with (
            tc.tile_pool(name="xs", bufs=6) as xs,
            tc.tile_pool(name="stats", bufs=1) as stats,
        ):
            NW = (KSL // P) * 2            # [P, 2048] w chunks
            NX = KT // 2                   # [P, 2, 1024] x chunks
            maxes = stats.tile([P, NW + NX], f32)
            for i in range(KSL // P):
                for h in range(2):
                    t = xs.tile([P, 2, 1024], f32, tag="stage", name="amax_w")
                    nc.sync.dma_start(
                        out=t, in_=wslt_v[:, i, h * 2048:(h + 1) * 2048]
                        .rearrange("p (a m) -> p a m", a=2))
                    nc.vector.tensor_reduce(
                        out=maxes[:, i * 2 + h:i * 2 + h + 1],
                        in_=t, axis=mybir.AxisListType.XY, op=mybir.AluOpType.max,
                        apply_absolute_value=True)
            for c in range(NX):
                t = xs.tile([P, 2, 1024], f32, tag="stage", name="amax_x")
                nc.sync.dma_start(out=t, in_=xt_v[:, 2 * c:2 * c + 2, :])
                nc.vector.tensor_reduce(
                    out=maxes[:, NW + c:NW + c + 1],
                    in_=t, axis=mybir.AxisListType.XY, op=mybir.AluOpType.max,
                    apply_absolute_value=True)

            pack = stats.tile([P, 2], f32)
            nc.vector.tensor_reduce(out=pack[:, 0:1], in_=maxes[:, 0:NW],
                                    axis=mybir.AxisListType.X, op=mybir.AluOpType.max)
            nc.vector.tensor_reduce(out=pack[:, 1:2], in_=maxes[:, NW:NW + NX],
                                    axis=mybir.AxisListType.X, op=mybir.AluOpType.max)
            packr = stats.tile([P, 2], f32)
            nc.gpsimd.partition_all_reduce(packr, pack, channels=P,
                                           reduce_op=bass_isa.ReduceOp.max)
            nc.sync.dma_start(out=pmax_ext[:], in_=packr[0:1, :])
    nc.compile()
    return nc


def build_graph_main(M=1024, N=4096, K=4096, ncores=NCORES):
    """NEFF-B: quantize + int8-exact matmul + epilogue; scales are inputs."""
    import concourse.bass as bass
    import concourse.mybir as mybir
    import concourse.tile as tile
    from concourse import bacc

    assert M % P == 0 and K % P == 0 and N % 512 == 0
    KT = K // P             # k tiles
    MB = M // P             # m blocks (output row strips)
    NB = N // 512           # n blocks of 512

    f32 = mybir.dt.float32
    bf16 = mybir.dt.bfloat16

    nc = bacc.Bacc(None, num_devices=ncores)

    xt_ext = nc.declare_dram_parameter("xT", [K, M], f32, isOutput=False)
    wt_ext = nc.declare_dram_parameter("wT", [K, N], f32, isOutput=False)
    sc_ext = nc.declare_dram_parameter("scales", [1, 4], f32, isOutput=False)
    b_ext = nc.declare_dram_parameter("bias", [N], f32, isOutput=False)
    out_ext = nc.declare_dram_parameter("out", [M, N], bf16, isOutput=True)

    xt_v = xt_ext[:].rearrange("(a p) m -> p a m", p=P)      # [P, KT, M]
    wt_v = wt_ext[:].rearrange("(a p) n -> p a n", p=P)      # [P, KT, N]
    o_v = out_ext[:].rearrange("(a p) n -> p a n", p=P)      # [P, MB, N]

    with tile.TileContext(nc) as tc:
        with (
            tc.tile_pool(name="xs", bufs=4) as xs,           # [P, 2, 1024] f32
            tc.tile_pool(name="ws", bufs=6) as ws,           # [P, 2, 512] f32
            tc.tile_pool(name="persist", bufs=1) as persist,
            tc.tile_pool(name="qwt", bufs=2) as qwtpool,     # [P, KT, 512] bf16
            tc.tile_pool(name="ob", bufs=4) as obpool,       # [P, 2, 512] bf16
            tc.tile_pool(name="stats", bufs=1) as stats,
            tc.tile_pool(name="psum_mm", bufs=8, space="PSUM") as psmm,
        ):
            # ---------- scales (host-computed) ----------
            sct = stats.tile([1, 4], f32)
            nc.sync.dma_start(out=sct, in_=sc_ext[:])
            sxb = stats.tile([P, 1], f32)
            swb = stats.tile([P, 1], f32)
            dsb = stats.tile([P, 1], f32)
            nc.gpsimd.partition_broadcast(sxb, sct[0:1, 0:1])
            nc.gpsimd.partition_broadcast(swb, sct[0:1, 1:2])
            nc.gpsimd.partition_broadcast(dsb, sct[0:1, 2:3])

            # bias replicated into all partitions (bf16)
            bias_t = persist.tile([P, N], bf16)
            bias_bcast = bass.AP(tensor=b_ext, offset=0, ap=[[0, P], [1, N]])
            nc.gpsimd.dma_start(out=bias_t, in_=bias_bcast)

            # ---------- w quantize: 2 k-tiles per DMA/DVE/ACT op ----------
            def wq_chunk(nb, qwT, kt):
                s = ws.tile([P, 2, 512], f32, tag="wstage", name="wstage")
                nc.sync.dma_start(
                    out=s, in_=wt_v[:, kt:kt + 2, nb * 512:(nb + 1) * 512])
                nc.vector.tensor_scalar(out=s, in0=s, scalar1=swb,
                                        scalar2=MAGIC, op0=mybir.AluOpType.mult,
                                        op1=mybir.AluOpType.add)
                nc.scalar.activation(out=qwT[:, kt:kt + 2, :], in_=s,
                                     func=mybir.ActivationFunctionType.Copy,
                                     bias=-MAGIC, scale=1.0)

            qw_tiles = {}
            qw_tiles[0] = qwtpool.tile([P, KT, 512], bf16, tag="qwt", name="qwt_blk")
            for kt in range(0, KT, 2):
                wq_chunk(0, qw_tiles[0], kt)

            # ---------- x quantize (2 k-tiles per op), w block 1 woven in ----------
            qxT = persist.tile([P, KT, M], bf16)
            for kt in range(0, KT, 2):
                t = xs.tile([P, 2, 1024], f32, tag="stage", name="xstage")
                nc.sync.dma_start(out=t, in_=xt_v[:, kt:kt + 2, :])
                nc.vector.tensor_scalar(out=t, in0=t, scalar1=sxb,
                                        scalar2=MAGIC, op0=mybir.AluOpType.mult,
                                        op1=mybir.AluOpType.add)
                nc.scalar.activation(out=qxT[:, kt:kt + 2, :], in_=t,
                                     func=mybir.ActivationFunctionType.Copy,
                                     bias=-MAGIC, scale=1.0)

            # ---------- matmul phase: w-quant spread across the matmuls ----------
            for nb in range(NB):
                qwT = qw_tiles.pop(nb)
                nxt = None
                if nb + 1 < NB:
                    qw_tiles[nb + 1] = qwtpool.tile([P, KT, 512], bf16,
                                                    tag="qwt", name="qwt_blk")
                    nxt = qw_tiles[nb + 1]
                accs = [psmm.tile([P, 512], f32, space="PSUM", name="acc")
                        for _ in range(MB)]
                if True:
                    for mb in range(MB):
                        for kt in range(KT):
                            nc.tensor.matmul(
                                accs[mb], qxT[:, kt, mb * P:(mb + 1) * P],
                                qwT[:, kt, :],
                                start=(kt == 0), stop=(kt == KT - 1))
                        if nxt is not None and mb % 2 == 1:
                            kt = (mb // 2) * (KT // 4)
                            wq_chunk(nb + 1, nxt, kt)
                            wq_chunk(nb + 1, nxt, kt + 2)
                            wq_chunk(nb + 1, nxt, kt + 4)
                            wq_chunk(nb + 1, nxt, kt + 6)
                for mb in range(0, MB, 2):
                    ob = obpool.tile([P, 2, 512], bf16, name="ob")
                    for j in range(2):
                        nc.vector.scalar_tensor_tensor(
                            out=ob[:, j, :], in0=accs[mb + j], scalar=dsb,
                            in1=bias_t[:, nb * 512:(nb + 1) * 512],
                            op0=mybir.AluOpType.mult, op1=mybir.AluOpType.add)
                    nc.sync.dma_start(
                        out=o_v[:, mb:mb + 2, nb * 512:(nb + 1) * 512],
                        in_=ob)
    nc.compile()
    return nc


def shard_inputs_amax(xT, wT, M=1024, K=4096, ncores=NCORES):
    ksl = K // ncores
    return [{
        "xT": np.ascontiguousarray(xT[:, c * M:(c + 1) * M]),
        "wslT": np.ascontiguousarray(wT[c * ksl:(c + 1) * ksl]),
    } for c in range(ncores)]


def _run(x, weight, bias, trace=False):
    from concourse.bass_utils import run_bass_kernel_spmd

    xf = np.asarray(x, dtype=np.float32).reshape(-1, x.shape[-1])
    xT = np.ascontiguousarray(xf.T)                                # [K, M_total]
    wT = np.ascontiguousarray(np.asarray(weight, dtype=np.float32).T)  # [K, N]
    b = np.ascontiguousarray(np.asarray(bias, dtype=np.float32))
    M = xT.shape[1] // NCORES

    nc_a = build_graph_amax()
    res_a = run_bass_kernel_spmd(nc_a, shard_inputs_amax(xT, wT),
                                 core_ids=list(range(NCORES)), trace=trace)
    pmax = np.stack([np.asarray(res_a.results[c]["pmax"], dtype=np.float32)
                     for c in range(NCORES)])                      # [NC, 1, 2]
    amax_w = np.float32(pmax[:, 0, 0].max())
    amax_x = np.float32(pmax[:, 0, 1].max())
    sx = np.float32(MAXV) / amax_x
    sw = np.float32(MAXV) / amax_w
    ds = np.float32(1.0) / (sx * sw)
    scales = np.array([[sx, sw, ds, 0.0]], dtype=np.float32)

    nc_b = build_graph_main()
    in_maps = [{
        "xT": np.ascontiguousarray(xT[:, c * M:(c + 1) * M]),
        "wT": wT,
        "scales": scales,
        "bias": b,
    } for c in range(NCORES)]
    res_b = run_bass_kernel_spmd(nc_b, in_maps, core_ids=list(range(NCORES)),
                                 trace=trace)
    outs = [np.asarray(res_b.results[c]["out"], dtype=np.float32)
            for c in range(NCORES)]
    full = np.concatenate(outs, axis=0).reshape(FULL_B, FULL_S, FULL_N)

    exec_a = res_a.exec_time_ns
    exec_b = res_b.exec_time_ns
    total = (exec_a or 0) + (exec_b or 0) if (exec_a or exec_b) else None

    class _Res:
        exec_time_ns = total
        exec_a_ns = exec_a
        exec_b_ns = exec_b
        results = res_b.results
    return full, _Res


def kernel(x, weight, bias):
    out, _ = _run(x, weight, bias, trace=False)
    return out


# revision 14
# speedup vs baseline: 1.1641x; 1.1641x over previous
"""AdaPT Linear (int8 systolic fake-quant matmul) on 8 TRN2 NeuronCores.

Reference semantics (single device):
    amax_x = max|x|, amax_w = max|w|         (global scalars)
    sx = 127/amax_x, sw = 127/amax_w
    qx = round(x*sx)  (int8), qw = round(w*sw)  (int8)
    out = (qx @ qw.T)_int32 / (sx*sw) + bias

Distribution: data-parallel over x rows (8 x 1024 rows per core).

Key structure (v5) — TWO small NEFFs, no on-device collective:
  An InstCollectiveCompute anywhere in a NEFF arms the CC machinery and
  measurably stretches EVERY matmul in that NEFF from ~216ns to ~266ns
  (cross-checked with microbenchmarks: the same matmul stream runs at
  216ns without a collective, 266ns with one, regardless of where the
  collective sits in the program).  So the global amax exchange is
  taken off-device:
    NEFF-A: each core reduces |x|-shard and |w|-k-slice partial maxima
            and writes a [1,2] partial to DRAM.  (pure DMA+DVE, ~85us)
    host:   max over the 8 partial pairs (16 floats) and the three
            scale scalars sx=127/amax_x, sw=127/amax_w, ds=1/(sx*sw) —
            the same f32 arithmetic the reference does.
    NEFF-B: quantize + matmul + epilogue with the scales as inputs —
            collective-free, so the PE runs at full rate.

  - host stages xT (k-major) per core and wT (k-major) replicated: NO
    on-chip transposes, both matmul operands load k-major.
  - quantization = fp32 magic-number round (v*s + 1.5*2^23, subtract
    back; fp32 RNE == round-half-even == jnp.round), 2 k-tiles per
    DVE/ACT op.
  - w-quant for block nb+2 is spread across the mb groups of block nb's
    matmuls; the qwT ring has 3 slots so those writes never alias the
    block the PE is reading.
  - matmul: lhsT = qxT k-tile [128k x 128m], rhs = qwT k-tile
    [128k x 512n], 32-step accumulation into fp32 PSUM (8 banks).
    int8 products (<2^14) and sums (<2^24) are exact in the bf16 PE
    datapath, reproducing the int8 MAC.
  - epilogue: out = psum * ds + bias in one DVE op, stored bf16
    (rel-err ~1e-3, well under the 2e-2 gate), 2 row-blocks per DMA;
    host casts back to f32.
"""

import numpy as np

P = 128
MAGIC = 12582912.0  # 1.5 * 2**23: fp32 RNE round-to-int trick
MAXV = 127.0
NCORES = 8

# full-problem shapes (hardcoded per the task)
FULL_B, FULL_S, FULL_K = 4, 2048, 4096
FULL_N = 4096


def build_graph_amax(M=1024, N=4096, K=4096, ncores=NCORES):
    """NEFF-A: per-core partial amax of the x shard and the w k-slice."""
    import concourse.mybir as mybir
    import concourse.tile as tile
    from concourse import bacc, bass_isa

    KT = K // P
    KSL = K // ncores

    f32 = mybir.dt.float32
    nc = bacc.Bacc(None, num_devices=ncores)

    xt_ext = nc.declare_dram_parameter("xT", [K, M], f32, isOutput=False)
    wslt_ext = nc.declare_dram_parameter("wslT", [KSL, N], f32, isOutput=False)
    pmax_ext = nc.declare_dram_parameter("pmax", [1, 2], f32, isOutput=True)

    xt_v = xt_ext[:].rearrange("(a p) m -> p a m", p=P)      # [P, KT, M]
    wslt_v = wslt_ext[:].rearrange("(a p) n -> p a n", p=P)  # [P, KSL/P, N]

    with tile.TileContext(nc) as tc:
        with (
            tc.tile_pool(name="xs", bufs=6) as xs,
            tc.tile_pool(name="stats", bufs=1) as stats,
        ):
            NW = (KSL // P) * 2            # [P, 2048] w chunks
            NX = KT // 2                   # [P, 2, 1024] x chunks
            maxes = stats.tile([P, NW + NX], f32)
            for i in range(KSL // P):
                for h in range(2):
                    t = xs.tile([P, 2, 1024], f32, tag="stage", name="amax_w")
                    nc.sync.dma_start(
                        out=t, in_=wslt_v[:, i, h * 2048:(h + 1) * 2048]
                        .rearrange("p (a m) -> p a m", a=2))
                    nc.vector.tensor_reduce(
                        out=maxes[:, i * 2 + h:i * 2 + h + 1],
                        in_=t, axis=mybir.AxisListType.XY, op=mybir.AluOpType.max,
                        apply_absolute_value=True)
            for c in range(NX):
                t = xs.tile([P, 2, 1024], f32, tag="stage", name="amax_x")
                nc.sync.dma_start(out=t, in_=xt_v[:, 2 * c:2 * c + 2, :])
                nc.vector.tensor_reduce(
                    out=maxes[:, NW + c:NW + c + 1],
                    in_=t, axis=mybir.AxisListType.XY, op=mybir.AluOpType.max,
                    apply_absolute_value=True)

            pack = stats.tile([P, 2], f32)
            nc.vector.tensor_reduce(out=pack[:, 0:1], in_=maxes[:, 0:NW],
                                    axis=mybir.AxisListType.X, op=mybir.AluOpType.max)
            nc.vector.tensor_reduce(out=pack[:, 1:2], in_=maxes[:, NW:NW + NX],
                                    axis=mybir.AxisListType.X, op=mybir.AluOpType.max)
            packr = stats.tile([P, 2], f32)
            nc.gpsimd.partition_all_reduce(packr, pack, channels=P,
                                           reduce_op=bass_isa.ReduceOp.max)
            nc.sync.dma_start(out=pmax_ext[:], in_=packr[0:1, :])
    nc.compile()
    return nc


def build_graph_main(M=1024, N=4096, K=4096, ncores=NCORES):
    """NEFF-B: quantize + int8-exact matmul + epilogue; scales are inputs."""
    import concourse.bass as bass
    import concourse.mybir as mybir
    import concourse.tile as tile
    from concourse import bacc

    assert M % P == 0 and K % P == 0 and N % 512 == 0
    KT = K // P             # k tiles
    MB = M // P             # m blocks (output row strips)
    NB = N // 512           # n blocks of 512

    f32 = mybir.dt.float32
    bf16 = mybir.dt.bfloat16

    nc = bacc.Bacc(None, num_devices=ncores)

    xt_ext = nc.declare_dram_parameter("xT", [K, M], f32, isOutput=False)
    wt_ext = nc.declare_dram_parameter("wT", [K, N], f32, isOutput=False)
    sc_ext = nc.declare_dram_parameter("scales", [1, 4], f32, isOutput=False)
    b_ext = nc.declare_dram_parameter("bias", [N], f32, isOutput=False)
    out_ext = nc.declare_dram_parameter("out", [M, N], bf16, isOutput=True)

    xt_v = xt_ext[:].rearrange("(a p) m -> p a m", p=P)      # [P, KT, M]
    wt_v = wt_ext[:].rearrange("(a p) n -> p a n", p=P)      # [P, KT, N]
    o_v = out_ext[:].rearrange("(a p) n -> p a n", p=P)      # [P, MB, N]

    with tile.TileContext(nc) as tc:
        with (
            tc.tile_pool(name="xs", bufs=4) as xs,           # [P, 2, 1024] f32
            tc.tile_pool(name="ws", bufs=6) as ws,           # [P, 2, 512] f32
            tc.tile_pool(name="persist", bufs=1) as persist,
            tc.tile_pool(name="qwt", bufs=2) as qwtpool,     # [P, KT, 512] bf16
            tc.tile_pool(name="ob", bufs=2) as obpool,       # [P, 2, 512] bf16
            tc.tile_pool(name="stats", bufs=1) as stats,
            tc.tile_pool(name="psum_mm", bufs=8, space="PSUM") as psmm,
        ):
            # ---------- scales (host-computed) ----------
            sct = stats.tile([1, 4], f32)
            nc.sync.dma_start(out=sct, in_=sc_ext[:])
            sxb = stats.tile([P, 1], f32)
            swb = stats.tile([P, 1], f32)
            dsb = stats.tile([P, 1], f32)
            nc.gpsimd.partition_broadcast(sxb, sct[0:1, 0:1])
            nc.gpsimd.partition_broadcast(swb, sct[0:1, 1:2])
            nc.gpsimd.partition_broadcast(dsb, sct[0:1, 2:3])

            # bias replicated into all partitions (bf16)
            bias_t = persist.tile([P, N], bf16)
            bias_bcast = bass.AP(tensor=b_ext, offset=0, ap=[[0, P], [1, N]])
            nc.gpsimd.dma_start(out=bias_t, in_=bias_bcast)

            # ---------- w quantize: 2 k-tiles per DMA/DVE/ACT op ----------
            def wq_chunk(nb, qwT, kt):
                s = ws.tile([P, 2, 512], f32, tag="wstage", name="wstage")
                nc.sync.dma_start(
                    out=s, in_=wt_v[:, kt:kt + 2, nb * 512:(nb + 1) * 512])
                nc.vector.tensor_scalar(out=s, in0=s, scalar1=swb,
                                        scalar2=MAGIC, op0=mybir.AluOpType.mult,
                                        op1=mybir.AluOpType.add)
                nc.scalar.activation(out=qwT[:, kt:kt + 2, :], in_=s,
                                     func=mybir.ActivationFunctionType.Copy,
                                     bias=-MAGIC, scale=1.0)

            qw_tiles = {}
            qw_tiles[0] = qwtpool.tile([P, KT, 512], bf16, tag="qwt", name="qwt_blk")
            for kt in range(0, KT, 2):
                wq_chunk(0, qw_tiles[0], kt)

            # ---------- x quantize (2 k-tiles per op), w block 1 woven in ----------
            qxT = persist.tile([P, KT, M], bf16)
            for kt in range(0, KT, 2):
                t = xs.tile([P, 2, 1024], f32, tag="stage", name="xstage")
                nc.sync.dma_start(out=t, in_=xt_v[:, kt:kt + 2, :])
                nc.vector.tensor_scalar(out=t, in0=t, scalar1=sxb,
                                        scalar2=MAGIC, op0=mybir.AluOpType.mult,
                                        op1=mybir.AluOpType.add)
                nc.scalar.activation(out=qxT[:, kt:kt + 2, :], in_=t,
                                     func=mybir.ActivationFunctionType.Copy,
                                     bias=-MAGIC, scale=1.0)

            # ---------- matmul phase: w-quant spread across the matmuls ----------
            for nb in range(NB):
                qwT = qw_tiles.pop(nb)
                nxt = None
                if nb + 1 < NB:
                    qw_tiles[nb + 1] = qwtpool.tile([P, KT, 512], bf16,
                                                    tag="qwt", name="qwt_blk")
                    nxt = qw_tiles[nb + 1]
                accs = [psmm.tile([P, 512], f32, space="PSUM", name="acc")
                        for _ in range(MB)]
                if True:
                    for mb in range(MB):
                        for kt in range(KT):
                            nc.tensor.matmul(
                                accs[mb], qxT[:, kt, mb * P:(mb + 1) * P],
                                qwT[:, kt, :],
                                start=(kt == 0), stop=(kt == KT - 1))
                        if nxt is not None and mb % 2 == 1:
                            kt = (mb // 2) * (KT // 4)
                            wq_chunk(nb + 1, nxt, kt)
                            wq_chunk(nb + 1, nxt, kt + 2)
                            wq_chunk(nb + 1, nxt, kt + 4)
                            wq_chunk(nb + 1, nxt, kt + 6)
                for mb in range(0, MB, 2):
                    ob = obpool.tile([P, 2, 512], bf16, name="ob")
                    for j in range(2):
                        nc.vector.scalar_tensor_tensor(
                            out=ob[:, j, :], in0=accs[mb + j], scalar=dsb,
                            in1=bias_t[:, nb * 512:(nb + 1) * 512],
                            op0=mybir.AluOpType.mult, op1=mybir.AluOpType.add)
                    nc.sync.dma_start(
                        out=o_v[:, mb:mb + 2, nb * 512:(nb + 1) * 512],
                        in_=ob)
    nc.compile()
    return nc


def shard_inputs_amax(xT, wT, M=1024, K=4096, ncores=NCORES):
    ksl = K // ncores
    return [{
        "xT": np.ascontiguousarray(xT[:, c * M:(c + 1) * M]),
        "wslT": np.ascontiguousarray(wT[c * ksl:(c + 1) * ksl]),
    } for c in range(ncores)]


def _run(x, weight, bias, trace=False):
    from concourse.bass_utils import run_bass_kernel_spmd

    xf = np.asarray(x, dtype=np.float32).reshape(-1, x.shape[-1])
    xT = np.ascontiguousarray(xf.T)                                # [K, M_total]
    wT = np.ascontiguousarray(np.asarray(weight, dtype=np.float32).T)  # [K, N]
    b = np.ascontiguousarray(np.asarray(bias, dtype=np.float32))
    M = xT.shape[1] // NCORES

    nc_a = build_graph_amax()
    res_a = run_bass_kernel_spmd(nc_a, shard_inputs_amax(xT, wT),
                                 core_ids=list(range(NCORES)), trace=trace)
    pmax = np.stack([np.asarray(res_a.results[c]["pmax"], dtype=np.float32)
                     for c in range(NCORES)])                      # [NC, 1, 2]
    amax_w = np.float32(pmax[:, 0, 0].max())
    amax_x = np.float32(pmax[:, 0, 1].max())
    sx = np.float32(MAXV) / amax_x
    sw = np.float32(MAXV) / amax_w
    ds = np.float32(1.0) / (sx * sw)
    scales = np.array([[sx, sw, ds, 0.0]], dtype=np.float32)

    nc_b = build_graph_main()
    in_maps = [{
        "xT": np.ascontiguousarray(xT[:, c * M:(c + 1) * M]),
        "wT": wT,
        "scales": scales,
        "bias": b,
    } for c in range(NCORES)]
    res_b = run_bass_kernel_spmd(nc_b, in_maps, core_ids=list(range(NCORES)),
                                 trace=trace)
    outs = [np.asarray(res_b.results[c]["out"], dtype=np.float32)
            for c in range(NCORES)]
    full = np.concatenate(outs, axis=0).reshape(FULL_B, FULL_S, FULL_N)

    exec_a = res_a.exec_time_ns
    exec_b = res_b.exec_time_ns
    total = (exec_a or 0) + (exec_b or 0) if (exec_a or exec_b) else None

    class _Res:
        exec_time_ns = total
        exec_a_ns = exec_a
        exec_b_ns = exec_b
        results = res_b.results
    return full, _Res


def kernel(x, weight, bias):
    out, _ = _run(x, weight, bias, trace=False)
    return out
